# revision 20
# baseline (speedup 1.0000x reference)
"""GQA attention (B=2,S=2048,D=1024,H=16,KH=4,HD=64) + RoPE + causal mask on 8 trn2 cores.

Sharding: core = (batch b, kv-group g).  Each core computes its 4 query heads'
attention against its single KV head and a partial output  O_g @ wo_g  [S, D]
in bf16; the host sums the 4 partials per batch in f32.

Per-core device pipeline (restructured from the 220us/249us baselines):
  - scores for the two heads of a GQA half-pair run as two CONCURRENT
    row-tiled K=64 matmuls (tile_position (0,0)/(64,0) via base partitions)
    into the two banks of one [128, 2, 512] PSUM tile; one exp ACTIVATE
    covers both heads (N=1024 amortizes the ~350cyc ACT overhead)
  - causal mask added inside score PSUM via (-BIG*I) @ stair matmuls against
    a single universal 128-col staircase block; diagonal k-tiles jj>=1 are
    column-restricted (fully-masked queries skipped in scores/mask/exp/AV)
  - AV pairs share the V lhsT; drains run two steps late so exp latency
    hides in the in-order PE stream
  - ScalarE exp (~1.15us per [128,1024] tile) is the attention rate limiter,
    so alternating non-diagonal tiles compute exp on the DVE instead as a
    ONE-INSTRUCTION Schraudolph bf16 exp:  int16(A*s + B) == bits(exp(s/8))
    (scores are bounded on non-diag tiles, so no clamping is needed);
    rel-err cost ~0.2-0.5% after the softmax average
  - the PE stream is kept dense with fine-grained FILLER: QKV-projection and
    wo-projection matmul chunks are threaded between attention steps from a
    deque, so the PE does not idle at ScalarE pace (idling re-throttles the
    PE clock gate to 1.2GHz: the baseline ran 82us cold)
  - normalization (1/denom from the ones-column of V): Ln then Exp(-x) on
    ScalarE at pair end (same ACT table set); the PE/DVE half is deferred
    into the next pair's first filler slots: r replicated across 64
    partitions with a K=1 outer-product matmul into PSUM (a stride-0-source
    DMA broadcast measured 5.8us and clogged the SWDGE queue), ScalarE hop
    to SBUF, one DVE mul.  j=0 lands straight in OTC; j=1 crosses
    partitions via one gpsimd DMA
  - input DMAs ride the sync HWDGE queue in consumer order (x block 0 in
    2-dc chunks; one queue is ~80GB/s and total HBM pull ~190GB/s, so only
    genuinely-parallel loads go elsewhere); the ACT queue carries the rope
    tables + x block 1; output stores ride sync, except the final block's
    which fan out across all three queues
  - partial outputs stored as bf16 (host sums in f32): halves store traffic
    and makes the PSUM evacuation a 2x-mode CAST

Measured (neuron-profile NTFF, core 0, min of 5): ~182-187us NEFF execution
on a quiet device (220us baseline; run-to-run spread from co-tenant device
load can reach +25%), rel err 7.2e-3.
"""

import os
import sys

import numpy as np

for _p in ("/opt/trn_rl_repo", "/root/.axon_site/_ro/trn_rl_repo"):
    if os.path.isdir(_p) and _p not in sys.path:
        sys.path.insert(0, _p)

from collections import deque
from contextlib import ExitStack

import concourse.bass as bass
import concourse.tile as tile
from concourse import mybir
from concourse.bass_utils import run_bass_kernel_spmd

B, S, D = 2, 2048, 1024
H, KH, HD = 16, 4, 64
REP = H // KH          # 4 query heads per kv head
GH = REP               # heads per core
P = 128
QB = 512               # q block (matmul moving free dim)
NKT = S // P           # 16 key tiles
NQB = S // QB          # 4 q blocks
DCH = D // P           # 8 contraction chunks for D
BIG = 30000.0          # pre-scale additive mask magnitude
# Schraudolph bf16 exp: bits(exp(x)) ~= 2^7/ln2 * x + 127*2^7 - C.  The score
# scale 1/8 is folded into the slope; C=5.5 centers the relative error
# (max ~3%, RMS ~1.9%) which washes out in the softmax average.
SCH_A = 184.664923 * 0.125
SCH_B = 16250.5

f32 = mybir.dt.float32
bf16 = mybir.dt.bfloat16

LAST_EXEC_NS = None
LAST_PROFILE = None


def _classify_mask(mask):
    m = np.asarray(mask).reshape(S, S)
    if not m.any():
        return "none"
    tril = np.tril(np.ones((S, S), dtype=bool))
    if (m[tril] == 0.0).all() and (m[~tril] < -1e30).all():
        return "causal"
    return "general"


def _build_nc(mode):
    nc = bass.Bass()
    xT = nc.declare_dram_parameter("xT", [NQB, P, DCH, QB], bf16, isOutput=False)
    wq = nc.declare_dram_parameter("wq", [P, DCH, GH * HD], bf16, isOutput=False)
    wk = nc.declare_dram_parameter("wk", [P, DCH, 2 * HD], bf16, isOutput=False)
    wv = nc.declare_dram_parameter("wv", [P, DCH, HD], bf16, isOutput=False)
    wo = nc.declare_dram_parameter("wo", [P, 2, D], bf16, isOutput=False)
    cos64 = nc.declare_dram_parameter("cos64", [64, S], bf16, isOutput=False)
    sin64 = nc.declare_dram_parameter("sin64", [64, S], bf16, isOutput=False)
    stair2 = nc.declare_dram_parameter("stair2", [P, P], bf16, isOutput=False)
    negI = nc.declare_dram_parameter("negI", [P, P], bf16, isOutput=False)
    if mode == "general":
        maskT = nc.declare_dram_parameter("maskT", [NKT, P, S], f32, isOutput=False)
    out = nc.declare_dram_parameter("out", [S, D], bf16, isOutput=True)

    with tile.TileContext(nc) as tc, ExitStack() as ctx:
        const = ctx.enter_context(tc.tile_pool(name="const", bufs=1))
        big = ctx.enter_context(tc.tile_pool(name="big", bufs=1))
        work = ctx.enter_context(tc.tile_pool(name="work", bufs=6))
        ptp = ctx.enter_context(tc.tile_pool(name="ptp", bufs=8))
        psp = ctx.enter_context(tc.tile_pool(name="psp", bufs=2, space="PSUM"))
        stp = ctx.enter_context(tc.tile_pool(name="stp", bufs=2, space="PSUM"))
        avp = ctx.enter_context(tc.tile_pool(name="avp", bufs=2, space="PSUM"))

        xt_sb = big.tile([P, NQB, DCH, QB], bf16, tag="xt")
        wq_sb = const.tile([P, DCH, GH * HD], bf16, tag="wq")
        wk_sb = const.tile([P, DCH, 2 * HD], bf16, tag="wk")
        wv_sb = const.tile([P, DCH, HD], bf16, tag="wv")
        wo_sb = const.tile([P, 2, D], bf16, tag="wo")
        cos_sb = const.tile([P, S], bf16, tag="cos")
        sin_sb = const.tile([P, S], bf16, tag="sin")
        stair2_sb = const.tile([P, P], bf16, tag="stair2")
        negI_sb = const.tile([P, P], bf16, tag="negI")

        # ---- input DMAs.  A DMA_DIRECT2D trigger costs ~600ns of ENGINE
        # time and one HWDGE queue moves ~80GB/s of a ~190GB/s aggregate:
        # sync carries the critical-path bulk in consumer order; the ACT
        # queue (idle after its trigger burst) carries the rope tables and
        # x block 1.
        nc.scalar.dma_start(out=cos_sb[0:64, :], in_=cos64[:, :])
        nc.scalar.dma_start(out=cos_sb[64:128, :], in_=cos64[:, :])
        nc.scalar.dma_start(out=sin_sb[0:64, :], in_=sin64[:, :])
        nc.scalar.dma_start(out=sin_sb[64:128, :], in_=sin64[:, :])
        # x1 rides the otherwise-idle ACT queue: on the sync queue it sat
        # ~2.4MB deep (~30us at ~80GB/s) and the A(1) projection fillers
        # stalled the PE 6us waiting for it
        nc.scalar.dma_start(out=xt_sb[:, 1, 0:4], in_=xT[1, :, 0:4])
        nc.scalar.dma_start(out=xt_sb[:, 1, 4:8], in_=xT[1, :, 4:8])
        nc.sync.dma_start(out=wk_sb, in_=wk[:, :, :])
        for d0 in range(0, DCH, 2):
            nc.sync.dma_start(out=xt_sb[:, 0, d0:d0 + 2],
                              in_=xT[0, :, d0:d0 + 2])
            if d0 == 0:
                nc.sync.dma_start(out=wq_sb, in_=wq[:, :, :])
        nc.sync.dma_start(out=wv_sb, in_=wv[:, :, :])
        nc.sync.dma_start(out=stair2_sb, in_=stair2[:, :])
        nc.sync.dma_start(out=negI_sb, in_=negI[:, :])
        nc.sync.dma_start(out=wo_sb, in_=wo[:, :, :])
        nc.sync.dma_start(out=xt_sb[:, 2], in_=xT[2])
        nc.sync.dma_start(out=xt_sb[:, 3], in_=xT[3])

        # per-s-block tiles
        QT_t = [big.tile([P, 2, QB], bf16, tag=f"QT{i}", name=f"QT{i}")
                for i in range(NQB)]
        KT_t = [big.tile([P, QB], bf16, tag=f"KT{i}", name=f"KT{i}")
                for i in range(NQB)]
        V_t = [big.tile([P, 4, HD + 1], bf16, tag=f"V{i}", name=f"V{i}")
               for i in range(NQB)]
        OTC_t = [big.tile([P, 2, QB], bf16, tag=f"OTC{i}", name=f"OTC{i}")
                 for i in range(NQB)]
        for i in range(NQB):
            nc.vector.memset(V_t[i][:, :, HD:HD + 1], 1.0)
        ones64 = const.tile([1, 64], bf16, tag="ones64")
        nc.vector.memset(ones64, 1.0)

        def rope(ps, out_ap, sl):
            # ps rows: per 64-group [evens(32), odds(32)]; swap 32-row halves.
            sb_ps = work.tile([P, QB], bf16, tag="ropesb")
            nc.vector.tensor_copy(sb_ps, ps)
            tmp = work.tile([P, QB], bf16, tag="ropetmp")
            for r0 in range(0, P, 64):
                nc.gpsimd.dma_start(
                    out=tmp[r0:r0 + 32, :], in_=sb_ps[r0 + 32:r0 + 64, :])
                nc.gpsimd.dma_start(
                    out=tmp[r0 + 32:r0 + 64, :], in_=sb_ps[r0:r0 + 32, :])
            ta = work.tile([P, QB], bf16, tag="ropeta")
            tb = work.tile([P, QB], bf16, tag="ropetb")
            nc.vector.tensor_mul(ta, sb_ps, cos_sb[:, sl])
            nc.vector.tensor_mul(tb, tmp, sin_sb[:, sl])
            nc.gpsimd.tensor_add(out_ap, ta, tb)

        # ---- stage A (QKV projections) as filler chunks (~0.5-1us PE each)
        def a_chunks(sb):
            sl = slice(sb * QB, (sb + 1) * QB)
            state = {}

            def kq_part(which, ch, dc0):
                def f():
                    key = (which, ch)
                    if dc0 == 0:
                        state[key] = psp.tile(
                            [P, QB], f32, tag="proj",
                            name=f"proj{sb}_{which}{ch}")
                    ps = state[key]
                    for dc in range(dc0, dc0 + 4):
                        lhsT = (wk_sb[:, dc, :] if which == "k"
                                else wq_sb[:, dc, ch * P:(ch + 1) * P])
                        nc.tensor.matmul(
                            ps, lhsT=lhsT, rhs=xt_sb[:, sb, dc, :],
                            start=(dc == 0), stop=(dc == DCH - 1))
                    if dc0 + 4 == DCH:
                        rope(ps, KT_t[sb] if which == "k"
                             else QT_t[sb][:, ch, :], sl)
                return f

            def v_part(st_i):
                def f():
                    ps = psp.tile([P, HD], f32, tag="proj",
                                  name=f"projv{sb}_{st_i}")
                    for dc in range(DCH):
                        nc.tensor.matmul(
                            ps,
                            lhsT=xt_sb[:, sb, dc, st_i * P:(st_i + 1) * P],
                            rhs=wv_sb[:, dc, :],
                            start=(dc == 0), stop=(dc == DCH - 1))
                    nc.vector.tensor_copy(V_t[sb][:, st_i, 0:HD], ps)
                return f

            main = [kq_part(w, c, d)
                    for (w, c) in (("k", 0), ("q", 0), ("q", 1))
                    for d in range(0, DCH, 4)]
            vparts = [v_part(i) for i in range(4)]
            return main, vparts

        # ---- wo projection chunks (bf16 partial out)
        def wo_chunks(qb):
            # the last block's stores (1MB) would serialize on the sync queue
            # after all compute; fan them across all three queues (the ACT
            # stream is idle by then, so its triggers are free)
            last = qb == NQB - 1
            rings = ([nc.sync, nc.scalar, nc.gpsimd] if last else [nc.sync])
            cs = []
            for qt in range(4 * qb, 4 * qb + 4):
                for dh in range(2):
                    def f(qt=qt, dh=dh):
                        ps = psp.tile([P, QB], f32, tag="proj",
                                      name=f"wops{qt}_{dh}")
                        for c in range(2):
                            nc.tensor.matmul(
                                ps,
                                lhsT=OTC_t[qt // 4][:, c,
                                                    (qt % 4) * P:(qt % 4 + 1) * P],
                                rhs=wo_sb[:, c, dh * 512:(dh + 1) * 512],
                                start=(c == 0), stop=(c == 1))
                        osb = work.tile([P, 512], bf16, tag="osb",
                                        name=f"osb{qt}_{dh}")
                        nc.vector.tensor_copy(osb, ps)
                        eng = rings[(qt * 2 + dh) % len(rings)]
                        eng.dma_start(
                            out=out[qt * P:(qt + 1) * P,
                                    dh * 512:(dh + 1) * 512], in_=osb)
                    cs.append(f)
            return cs

        # ---- filler machinery
        fq = deque()

        def filler(n=1):
            for _ in range(n):
                if fq:
                    fq.popleft()[1]()

        def flush_tag(tag):
            kept = []
            while fq:
                t, c = fq.popleft()
                if t == tag:
                    c()
                else:
                    kept.append((t, c))
            fq.extend(kept)

        def _normalize(qb, ch, avs):
            # 1/denom as exp(-ln d) on ScalarE (same ACT table set).  The
            # r-replication across 64 partitions uses a stride-0-source DMA
            # broadcast on the otherwise-idle ACT HWDGE ring: its ~3-6us
            # transfer latency is fully hidden because wo fillers ride two
            # blocks behind, and unlike the K=1 outer-product matmul it puts
            # ZERO work in the in-order PE stream (the rbs matmuls stalled
            # ~2us at every pair boundary waiting on the ACT chain).  av is
            # evacuated to SBUF immediately so the next pair's AV reuse of
            # the PSUM bank unblocks after ~1us; the multiply is deferred to
            # the back of the filler queue (popped once rbs has landed).
            # The last pair keeps the matmul path (the PE is idle in the
            # tail, and the broadcast latency would sit on the exit chain);
            # non-causal modes run no fillers, so they use it too.
            use_bcast = mode == "causal" and not (qb == NQB - 1 and ch == 1)
            for j in (0, 1):
                avj = avs[j]
                lt = work.tile([1, QB], f32, tag="lnt",
                               name=f"lnt{qb}_{ch}_{j}")
                nc.scalar.activation(
                    lt, avj[HD:HD + 1, :], mybir.ActivationFunctionType.Ln)
                r1 = work.tile([1, QB], bf16, tag="r1",
                               name=f"r1_{qb}_{ch}_{j}")
                nc.scalar.activation(
                    r1, lt, mybir.ActivationFunctionType.Exp, scale=-1.0)
                if use_bcast:
                    av_sb = work.tile([64, QB], bf16, tag="avsb",
                                      name=f"avsb{qb}_{ch}_{j}")
                    nc.vector.tensor_copy(av_sb, avj[0:HD, :])
                    rbs_sb = work.tile([64, QB], bf16, tag="rbs",
                                       name=f"rbssb{qb}_{ch}_{j}")
                    r1b = bass.AP(tensor=r1.tensor, offset=r1.offset,
                                  ap=[list(r1.ap[0]), [0, 64]]
                                  + [list(a) for a in r1.ap[1:]])
                    with nc.allow_non_contiguous_dma(
                            reason="partition broadcast"):
                        nc.scalar.dma_start(out=rbs_sb, in_=r1b)

                    def mul_cl(j=j, av_sb=av_sb, rbs_sb=rbs_sb):
                        if j == 0:
                            nc.vector.tensor_mul(
                                OTC_t[qb][0:64, ch, :], av_sb, rbs_sb)
                        else:
                            ot = work.tile([64, QB], bf16, tag="ot",
                                           name=f"ot{qb}_{ch}_{j}")
                            nc.vector.tensor_mul(ot, av_sb, rbs_sb)
                            nc.gpsimd.dma_start(
                                out=OTC_t[qb][64:128, ch, :], in_=ot)
                    fq.append(("M", mul_cl))
                else:
                    def phase2(j=j, avj=avj, r1=r1):
                        rbs = psp.tile([64, QB], f32, tag="proj",
                                       name=f"rbs{qb}_{ch}_{j}")
                        nc.tensor.matmul(rbs, lhsT=ones64, rhs=r1,
                                         start=True, stop=True)
                        rbs_sb = work.tile([64, QB], f32, tag="rbsf",
                                           name=f"rbssf{qb}_{ch}_{j}")
                        nc.scalar.copy(rbs_sb, rbs)
                        if j == 0:
                            nc.vector.tensor_mul(
                                OTC_t[qb][0:64, ch, :], avj[0:HD, :], rbs_sb)
                        else:
                            ot = work.tile([64, QB], bf16, tag="ot",
                                           name=f"ot{qb}_{ch}_{j}")
                            nc.vector.tensor_mul(ot, avj[0:HD, :], rbs_sb)
                            nc.gpsimd.dma_start(
                                out=OTC_t[qb][64:128, ch, :], in_=ot)
                    fq.appendleft(("N", phase2))

        # ---- attention for one (q block, head pair); scores for the two
        # heads run as concurrent row-tiled K=64 matmuls.
        def emit_attn_pair(qb, ch):
            q0 = qb * QB
            qsl = slice(q0, q0 + QB)
            nk = 4 * (qb + 1) if mode == "causal" else NKT
            av = [avp.tile([HD + 1, QB], f32, tag="av",
                           name=f"av{qb}_{ch}_{j}") for j in (0, 1)]
            pend = []

            def drain(item):
                pt_, c0_, kt_ = item
                for j in (0, 1):
                    nc.tensor.matmul(
                        av[j][:, c0_:], lhsT=V_t[kt_ // 4][:, kt_ % 4, :],
                        rhs=pt_[:, j, c0_:],
                        start=(kt_ == 0), stop=(kt_ == nk - 1))
                if kt_ == nk - 1:
                    _normalize(qb, ch, av)

            for kt in range(nk):
                diag = mode == "causal" and kt >= nk - 4
                jj = kt - (nk - 4)
                c0 = 128 * jj if (diag and jj >= 1) else 0
                st = stp.tile([P, 2, QB], f32, tag="st",
                              name=f"st{qb}_{ch}_{kt}")
                for j in (0, 1):
                    hr = j * 64
                    nc.tensor.matmul(
                        st[:, j, c0:],
                        lhsT=KT_t[kt // 4][hr:hr + 64,
                                           (kt % 4) * P:(kt % 4 + 1) * P],
                        rhs=QT_t[qb][hr:hr + 64, ch, c0:],
                        start=True, stop=not diag)
                if diag:
                    mq0 = 128 * jj
                    for j in (0, 1):
                        nc.tensor.matmul(
                            st[:, j, mq0:mq0 + 128], lhsT=negI_sb,
                            rhs=stair2_sb, start=False, stop=True)
                if mode == "general":
                    mt = work.tile([P, QB], f32, tag="maskt")
                    nc.sync.dma_start(out=mt, in_=maskT[kt, :, qsl])
                    for j in (0, 1):
                        nc.vector.tensor_add(st[:, j, :], st[:, j, :], mt)
                pt = ptp.tile([P, 2, QB], bf16, tag="pt",
                              name=f"pt{qb}_{ch}_{kt}")
                # exp: ScalarE ACTIVATE is the attention rate limiter (~1.15us
                # per [128,1024] tile); offload alternating non-diagonal tiles
                # to the DVE as a one-op Schraudolph bf16 exp:
                #   bits(exp(s/8)) ~= int16(SCH_A*s + SCH_B)
                # (safe only when scores are bounded: causal non-diag tiles,
                # where s/8 in [-30,30] keeps the affine in int16 range).
                if mode == "causal" and not diag and kt % 2 == 1:
                    nc.vector.tensor_scalar(
                        pt.bitcast(mybir.dt.int16), st,
                        SCH_A, SCH_B,
                        mybir.AluOpType.mult, mybir.AluOpType.add)
                else:
                    nc.scalar.activation(
                        pt[:, :, c0:], st[:, :, c0:],
                        mybir.ActivationFunctionType.Exp, scale=0.125)
                pend.append((pt, c0, kt))
                if len(pend) > 2:
                    drain(pend.pop(0))
                filler(2 if kt < 3 else 1)
            while pend:
                drain(pend.pop(0))

        # ---- schedule
        if mode == "causal":
            m0, v0 = a_chunks(0)
            for c in m0[0:4]:   # K, Q ch0
                c()
            for c in v0:        # V
                c()
            for c in m0[4:6]:   # Q ch1
                c()
            pend_v = {}
            for qb in range(NQB):
                if qb + 1 < NQB:
                    mn, vp = a_chunks(qb + 1)
                    fq.extend((f"A{qb + 1}", c) for c in mn)
                    pend_v[qb + 1] = vp
                # wo fillers ride two blocks behind: attn(3) is the
                # longest ScalarE-paced stretch and otherwise runs out of
                # filler, letting the PE clock-gate re-throttle (~10us cold
                # stretches observed there)
                if qb - 2 >= 0:
                    fq.extend((f"W{qb - 2}", c) for c in wo_chunks(qb - 2))
                if qb == NQB - 1:
                    fq.extend((f"W{qb - 1}", c) for c in wo_chunks(qb - 1))
                for ch in (0, 1):
                    emit_attn_pair(qb, ch)
                if qb + 1 < NQB:
                    # QKV of the next block must be fully emitted before its
                    # attention; V units go near the FRONT (needed only for
                    # the last 4 k-tiles) but BEHIND any pending deferred
                    # normalize phase2 ("N") closures: those must emit before
                    # the next pair's first AV reuses the av PSUM slots.
                    flush_tag(f"A{qb + 1}")
                    ns = []
                    while fq and fq[0][0] == "N":
                        ns.append(fq.popleft())
                    for c in reversed(pend_v[qb + 1]):
                        fq.appendleft((f"V{qb + 1}", c))
                    for n in reversed(ns):
                        fq.appendleft(n)
            while fq:
                fq.popleft()[1]()
            for c in wo_chunks(NQB - 1):
                c()
        else:
            # non-causal attn reads ALL KT/V tiles; emit in phase order
            m0, v0 = a_chunks(0)
            for c in m0[0:4] + v0 + m0[4:6]:
                c()
            for sb in range(1, NQB):
                mn, vp = a_chunks(sb)
                for c in mn[0:4] + vp + mn[4:6]:
                    c()
            for qb in range(NQB):
                for ch in (0, 1):
                    emit_attn_pair(qb, ch)
            for qb in range(NQB):
                for c in wo_chunks(qb):
                    c()
    # split multi-wait conditions: TRN2 instructions hold at most one sync
    # wait (EventSemaphore holds two); walrus refuses to split them itself
    import bass_rust
    bass_rust.move_matmul_waits_to_ldweights(nc.m)
    bass_rust.generate_event_semaphores(nc)
    return nc


_NC_CACHE = {}


def _install_ntff_hook():
    """Best-effort: register the axon NTFF profile hook so trace=True can
    measure true NEFF execution time. Harmless no-op when unavailable."""
    try:
        import types
        if "antenv.axon_hooks" in sys.modules:
            return True
        import antenv
        mod = types.ModuleType("antenv.axon_hooks")
        mod._hook = None
        def set_axon_ntff_profile_hook(h):
            mod._hook = h
        def get_axon_ntff_profile_hook():
            return mod._hook
        mod.set_axon_ntff_profile_hook = set_axon_ntff_profile_hook
        mod.get_axon_ntff_profile_hook = get_axon_ntff_profile_hook
        from trn_agent_boot.trn_boot import _ntff_profile_via_ctypes
        hook = _ntff_profile_via_ctypes('/opt/axon/libaxon_pjrt.so')
        if hook is None:
            return False
        mod.set_axon_ntff_profile_hook(hook)
        sys.modules["antenv.axon_hooks"] = mod
        antenv.axon_hooks = mod
        return True
    except Exception:
        return False


def kernel(_trace=False, _trace_cores=None, **inputs):
    global LAST_EXEC_NS, LAST_PROFILE
    x = np.ascontiguousarray(np.asarray(inputs["x"], dtype=np.float32))
    wq = np.asarray(inputs["wq"], dtype=np.float32)
    wk = np.asarray(inputs["wk"], dtype=np.float32)
    wv = np.asarray(inputs["wv"], dtype=np.float32)
    wo = np.asarray(inputs["wo"], dtype=np.float32)
    fc = np.asarray(inputs["freqs_cos"], dtype=np.float32)
    fs = np.asarray(inputs["freqs_sin"], dtype=np.float32)
    mask = np.asarray(inputs["mask"], dtype=np.float32)

    mode = _classify_mask(mask)
    if mode not in _NC_CACHE:
        _NC_CACHE[mode] = _build_nc(mode)
    nc = _NC_CACHE[mode]
    in_maps = _make_in_maps(x, wq, wk, wv, wo, fc, fs, mask, mode)

    if _trace:
        _install_ntff_hook()
    kw = {"trace_cores": _trace_cores} if _trace_cores else {}
    try:
        res = run_bass_kernel_spmd(
            nc, in_maps, core_ids=list(range(8)), trace=_trace, **kw)
    except (ModuleNotFoundError, ImportError):
        res = run_bass_kernel_spmd(
            nc, in_maps, core_ids=list(range(8)), trace=False)
    LAST_EXEC_NS = res.exec_time_ns
    LAST_PROFILE = res.profile_json
    full = np.zeros((B, S, D), dtype=np.float32)
    for b in range(B):
        for g in range(KH):
            full[b] += np.asarray(res.results[b * KH + g]["out"],
                                  dtype=np.float32)
    return full


def _make_in_maps(x, wq, wk, wv, wo, fc, fs, mask, mode):
    # head_dim permutation: evens then odds (consistent on q & k -> scores invariant)
    perm = np.concatenate([np.arange(0, HD, 2), np.arange(1, HD, 2)])
    wq_p = wq.reshape(D, H, HD)[:, :, perm].reshape(D, H * HD)
    wk_p = wk.reshape(D, KH, HD)[:, :, perm].reshape(D, KH * HD)

    cosT = fc.T.astype(np.float32)                      # [32, S]
    sinT = fs.T.astype(np.float32)
    cos64 = np.ascontiguousarray(np.tile(cosT, (2, 1)))            # [64, S]
    sin64 = np.ascontiguousarray(np.concatenate([-sinT, sinT], axis=0))

    cc = np.arange(P)[:, None]
    tt = np.arange(P)[None, :]
    stair2 = (cc > tt).astype(np.float32)                          # [128,128]
    negI = (-BIG * np.eye(P)).astype(np.float32)

    import ml_dtypes
    b16 = ml_dtypes.bfloat16

    def _pcf(w_slice, width):
        # [D, width] -> [P, DCH, width]: partition-contiguous weight layout
        a = np.ascontiguousarray(w_slice).reshape(DCH, P, width)
        return np.ascontiguousarray(a.transpose(1, 0, 2))

    in_maps = []
    for b in range(B):
        # x[b].T [D, S] -> [sb, p, c, q] so each s-block DMA is contiguous
        xTb = np.ascontiguousarray(
            x[b].T.reshape(DCH, P, NQB, QB).transpose(2, 1, 0, 3)
        ).astype(b16)
        for g in range(KH):
            wk_g = wk_p[:, g * HD:(g + 1) * HD]
            wk_dup = np.concatenate([wk_g, wk_g], axis=1)       # [D, 128]
            wo_g = wo[g * GH * HD:(g + 1) * GH * HD].reshape(2, P, D)
            m = {
                "xT": xTb,
                "wq": _pcf(wq_p[:, g * GH * HD:(g + 1) * GH * HD],
                           GH * HD).astype(b16),
                "wk": _pcf(wk_dup, 2 * HD).astype(b16),
                "wv": _pcf(wv[:, g * HD:(g + 1) * HD], HD).astype(b16),
                "wo": np.ascontiguousarray(
                    wo_g.transpose(1, 0, 2)).astype(b16),
                "cos64": cos64.astype(b16),
                "sin64": sin64.astype(b16),
                "stair2": stair2.astype(b16),
                "negI": negI.astype(b16),
            }
            if mode == "general":
                m["maskT"] = np.ascontiguousarray(
                    mask.reshape(S, S).T).reshape(NKT, P, S)
            in_maps.append(m)
    return in_maps
